# revision 14
# baseline (speedup 1.0000x reference)
"""Bass/Tile TRN2 kernel for nn_CutlassLinear (int8-quantized linear, 4096x4096x4096).

Math (matches the reference):
    scale = 127 / max|W|
    w_q   = clip(trunc(W * scale), -127, 127)        # exact small ints
    y     = (x @ w_q^T) * (1/scale) + bias

Distribution: data-parallel over the 4096 token rows -- each of the 8
NeuronCores computes 512 token rows against the full weight matrix. No
collectives; outputs are disjoint row blocks.

Device kernel (per core, SPMD):
  - w_q is held in bf16 (integer values <= 127 are exact in bf16) and
    streamed through SBUF once as the stationary matmul operand.
  - x arrives transposed ([in, tok] layout) in fp32, is converted to bf16
    on-device (DVE) and pinned in SBUF for the whole kernel.
  - PE accumulates over the 4096-deep contraction in PSUM (fp32), 32
    matmuls of [128k x 128m] @ [128k x 512n] per output block.
  - PSUM eviction is fused with dequant scale + bias on the scalar engine:
    out = psum * (1/scale) + bias.
"""

import numpy as np
import ml_dtypes

P = 128
N_TOKENS = 4096
IN_F = 4096
OUT_F = 4096
N_CORES = 8
TOK = N_TOKENS // N_CORES  # 512 tokens per core
KO = IN_F // P             # 32 contraction blocks
MO = OUT_F // P            # 32 output-feature blocks

BF16 = ml_dtypes.bfloat16


def build_program(debug=False):
    import concourse.mybir as mybir
    import concourse.tile as tile
    from concourse import bacc

    f32 = mybir.dt.float32
    bf16 = mybir.dt.bfloat16

    nc = bacc.Bacc("TRN2", target_bir_lowering=False, debug=debug,
                   num_devices=N_CORES)

    xT = nc.dram_tensor("xT", [P, KO, TOK], f32, kind="ExternalInput").ap()
    wq = nc.dram_tensor("wq", [MO, P, KO, P], mybir.dt.int8,
                        kind="ExternalInput").ap()
    bias = nc.dram_tensor("bias", [P, MO], f32, kind="ExternalInput").ap()
    inv_s = nc.dram_tensor("inv_s", [P, 1], f32, kind="ExternalInput").ap()
    yT = nc.dram_tensor("yT", [P, MO, TOK], f32, kind="ExternalOutput").ap()

    # x load chunk widths in ko blocks (each block = [128, 512] f32).
    # Small first chunks let the PE start early; the sum must be KO.
    CHUNKS = [2, 2, 4, 4, 4, 4, 4, 4, 4]
    assert sum(CHUNKS) == KO
    NCH = len(CHUNKS)
    W_PREFETCH = 3
    SPLIT = 4         # head groups computed chunk-major during the x load

    with tile.TileContext(nc) as tc:
        with (
            tc.tile_pool(name="const", bufs=1) as const,
            tc.tile_pool(name="xbf", bufs=1) as xpool,
            tc.tile_pool(name="xstage", bufs=5) as xstage,
            tc.tile_pool(name="wstage", bufs=4) as wstage,
            tc.tile_pool(name="wpool", bufs=SPLIT + W_PREFETCH) as wpool,
            tc.tile_pool(name="opool", bufs=4) as opool,
            tc.tile_pool(name="psh", bufs=1, space="PSUM") as pshead,
            tc.tile_pool(name="ps", bufs=4, space="PSUM") as pspool,
        ):
            bias_sb = const.tile([P, MO], f32)
            nc.sync.dma_start(out=bias_sb[:], in_=bias)
            scale_sb = const.tile([P, 1], f32)
            nc.sync.dma_start(out=scale_sb[:], in_=inv_s)

            wt_tiles = [None] * MO

            def load_w(mo):
                # int8 on the wire (half the bytes); upcast to bf16 on the
                # otherwise-idle gpsimd engine (1-input copy = line rate).
                ws = wstage.tile([P, KO, P], mybir.dt.int8, name="ws")
                nc.sync.dma_start(out=ws[:], in_=wq[mo])
                wt = wpool.tile([P, KO, P], bf16, name="wt")
                nc.gpsimd.tensor_copy(out=wt[:], in_=ws[:])
                wt_tiles[mo] = wt

            def evict(mo, ps):
                ot = opool.tile([P, TOK], f32, name="ot")
                nc.scalar.activation(
                    ot[:], ps[:], mybir.ActivationFunctionType.Identity,
                    bias=bias_sb[:, mo:mo + 1], scale=scale_sb[:, 0:1],
                )
                nc.gpsimd.dma_start(out=yT[:, mo, :], in_=ot[:])

            # x: fp32 [in, tok] -> bf16, pinned in SBUF (32KB/partition).
            # x chunks alternate between the two DMA paths (weights share
            # the sync path) so the streams run in parallel.
            chunk_start = [sum(CHUNKS[:i]) for i in range(NCH)]
            x_tiles = []
            load_w(0)
            w_emitted = 1
            for c in range(NCH):
                cw = CHUNKS[c]
                xt = xpool.tile([P, cw, TOK], bf16, name=f"xbf{c}")
                st = xstage.tile([P, 4, TOK], f32, name="xst")[:, :cw, :]
                eng = nc.gpsimd if c % 2 == 0 else nc.sync
                eng.dma_start(
                    out=st[:], in_=xT[:, chunk_start[c]:chunk_start[c] + cw, :])
                nc.vector.tensor_copy(out=xt[:], in_=st[:])
                x_tiles.append(xt)
                if w_emitted < SPLIT:
                    load_w(w_emitted)
                    w_emitted += 1
            for mo in range(w_emitted, SPLIT + W_PREFETCH):
                load_w(mo)

            def x_block(ko):
                for c in range(NCH):
                    if chunk_start[c] <= ko < chunk_start[c] + CHUNKS[c]:
                        return x_tiles[c][:, ko - chunk_start[c], :]
                raise AssertionError(ko)

            # Head: SPLIT open PSUM groups accumulated chunk-major, so the
            # PE consumes each x chunk the moment it lands instead of
            # stalling until the whole 8MB x transfer completes.
            ps_head = [pshead.tile([P, TOK], mybir.dt.float32, name=f"psh{m}")
                       for m in range(SPLIT)]
            for c in range(NCH):
                for mo in range(SPLIT):
                    for j in range(CHUNKS[c]):
                        ko = chunk_start[c] + j
                        nc.tensor.matmul(
                            ps_head[mo][:],
                            lhsT=wt_tiles[mo][:, ko, :],
                            rhs=x_tiles[c][:, j, :],
                            start=(ko == 0),
                            stop=(ko == KO - 1),
                        )
            for mo in range(SPLIT):
                evict(mo, ps_head[mo])
                wt_tiles[mo] = None

            # Steady state: one group per mo, K-contiguous.
            for mo in range(SPLIT, MO):
                if mo + W_PREFETCH < MO:
                    load_w(mo + W_PREFETCH)
                wt = wt_tiles[mo]
                ps = pspool.tile([P, TOK], mybir.dt.float32, name="ps")
                for ko in range(KO):
                    nc.tensor.matmul(
                        ps[:],
                        lhsT=wt[:, ko, :],
                        rhs=x_block(ko),
                        start=(ko == 0),
                        stop=(ko == KO - 1),
                    )
                wt_tiles[mo] = None
                evict(mo, ps)

    nc.compile()
    return nc


def prep_inputs(x, weight, bias):
    """Host-side shard/layout prep. Returns (in_maps, inv_scale)."""
    x = np.asarray(x, dtype=np.float32)
    weight = np.asarray(weight, dtype=np.float32)
    bias = np.asarray(bias, dtype=np.float32)

    # Quantize weights exactly as the reference does (fp32 arithmetic).
    s = np.float32(127.0) / np.max(np.abs(weight))
    wq_f = np.clip(np.trunc(weight * s), -127.0, 127.0)
    inv_scale = np.float32(1.0) / s

    # w_q^T laid out [mo, p(k), ko, q(out)] so each per-core DMA block
    # [p, ko, q] is contiguous per partition. int8 (exact): upcast on device.
    wq_i8 = wq_f.astype(np.int8)
    wq_dram = np.ascontiguousarray(
        wq_i8.reshape(MO, P, KO, P).transpose(0, 3, 2, 1)
    )

    bias_dram = np.ascontiguousarray(bias.reshape(MO, P).T)
    inv_dram = np.full((P, 1), inv_scale, dtype=np.float32)

    in_maps = []
    for c in range(N_CORES):
        x_c = x[c * TOK:(c + 1) * TOK, :]                    # [tok, in]
        xT_dram = np.ascontiguousarray(
            x_c.reshape(TOK, KO, P).transpose(2, 1, 0))      # [p, ko, tok]
        in_maps.append({
            "xT": xT_dram,
            "wq": wq_dram,
            "bias": bias_dram,
            "inv_s": inv_dram,
        })
    return in_maps


def gather_output(results):
    """results: list of per-core dicts with 'yT' [P, MO, TOK] -> y [4096, 4096]."""
    blocks = []
    for c in range(N_CORES):
        yT = results[c]["yT"]                                # [q, mo, tok]
        y_c = yT.transpose(1, 0, 2).reshape(OUT_F, TOK).T    # [tok, out]
        blocks.append(y_c)
    return np.ascontiguousarray(np.concatenate(blocks, axis=0), dtype=np.float32)


_NC_CACHE = None


def get_program():
    global _NC_CACHE
    if _NC_CACHE is None:
        _NC_CACHE = build_program(debug=False)
    return _NC_CACHE


def run(x, weight, bias, trace=False, **run_kwargs):
    from concourse.bass_utils import run_bass_kernel_spmd

    nc = get_program()
    in_maps = prep_inputs(x, weight, bias)
    res = run_bass_kernel_spmd(nc, in_maps, list(range(N_CORES)),
                               trace=trace, **run_kwargs)
    return gather_output(res.results), res


def kernel(x, weight, bias):
    y, _ = run(x, weight, bias, trace=False)
    return y


# revision 15
# speedup vs baseline: 2.0110x; 2.0110x over previous
"""Bass/Tile TRN2 kernel for nn_CutlassLinear (int8-quantized linear, 4096x4096x4096).

Math (matches the reference):
    scale = 127 / max|W|
    w_q   = clip(trunc(W * scale), -127, 127)        # exact small ints
    y     = (x @ w_q^T) * (1/scale) + bias

Distribution: data-parallel over the 4096 token rows -- each of the 8
NeuronCores computes 512 token rows against the full weight matrix. No
collectives; outputs are disjoint row blocks.

Device kernel (per core, SPMD):
  - w_q is held in bf16 (integer values <= 127 are exact in bf16) and
    streamed through SBUF once as the stationary matmul operand.
  - x arrives transposed ([in, tok] layout) in fp32, is converted to bf16
    on-device (DVE) and pinned in SBUF for the whole kernel.
  - PE accumulates over the 4096-deep contraction in PSUM (fp32), 32
    matmuls of [128k x 128m] @ [128k x 512n] per output block.
  - PSUM eviction is fused with dequant scale + bias on the scalar engine:
    out = psum * (1/scale) + bias.
"""

import numpy as np
import ml_dtypes

P = 128
N_TOKENS = 4096
IN_F = 4096
OUT_F = 4096
N_CORES = 8
TOK = N_TOKENS // N_CORES  # 512 tokens per core
KO = IN_F // P             # 32 contraction blocks
MO = OUT_F // P            # 32 output-feature blocks

BF16 = ml_dtypes.bfloat16


def build_program(debug=False):
    import concourse.mybir as mybir
    import concourse.tile as tile
    from concourse import bacc

    f32 = mybir.dt.float32
    bf16 = mybir.dt.bfloat16

    nc = bacc.Bacc("TRN2", target_bir_lowering=False, debug=debug,
                   num_devices=N_CORES)

    xT = nc.dram_tensor("xT", [P, KO, TOK], f32, kind="ExternalInput").ap()
    wq = nc.dram_tensor("wq", [MO, P, KO, P], mybir.dt.int8,
                        kind="ExternalInput").ap()
    bias = nc.dram_tensor("bias", [P, MO], f32, kind="ExternalInput").ap()
    inv_s = nc.dram_tensor("inv_s", [P, 1], f32, kind="ExternalInput").ap()
    yT = nc.dram_tensor("yT", [P, MO, TOK], f32, kind="ExternalOutput").ap()

    # x load chunk widths in ko blocks (each block = [128, 512] f32).
    # Small first chunks let the PE start early; the sum must be KO.
    CHUNKS = [2, 2, 4, 4, 4, 4, 4, 4, 4]
    assert sum(CHUNKS) == KO
    NCH = len(CHUNKS)
    W_PREFETCH = 3
    SPLIT = 4         # head groups computed chunk-major during the x load

    with tile.TileContext(nc) as tc:
        with (
            tc.tile_pool(name="const", bufs=1) as const,
            tc.tile_pool(name="xbf", bufs=1) as xpool,
            tc.tile_pool(name="xstage", bufs=5) as xstage,
            tc.tile_pool(name="wstage", bufs=4) as wstage,
            tc.tile_pool(name="wpool", bufs=SPLIT + W_PREFETCH) as wpool,
            tc.tile_pool(name="opool", bufs=4) as opool,
            tc.tile_pool(name="psh", bufs=1, space="PSUM") as pshead,
            tc.tile_pool(name="ps", bufs=4, space="PSUM") as pspool,
        ):
            bias_sb = const.tile([P, MO], f32)
            nc.sync.dma_start(out=bias_sb[:], in_=bias)
            scale_sb = const.tile([P, 1], f32)
            nc.sync.dma_start(out=scale_sb[:], in_=inv_s)

            wt_tiles = [None] * MO

            def load_w(mo):
                # int8 on the wire (half the bytes); upcast to bf16 on DVE
                # (~2.5us/tile, well under the 6.9us per-group PE budget).
                ws = wstage.tile([P, KO, P], mybir.dt.int8, name="ws")
                nc.sync.dma_start(out=ws[:], in_=wq[mo])
                wt = wpool.tile([P, KO, P], bf16, name="wt")
                nc.vector.tensor_copy(out=wt[:], in_=ws[:])
                wt_tiles[mo] = wt

            def evict(mo, ps):
                ot = opool.tile([P, TOK], f32, name="ot")
                nc.scalar.activation(
                    ot[:], ps[:], mybir.ActivationFunctionType.Identity,
                    bias=bias_sb[:, mo:mo + 1], scale=scale_sb[:, 0:1],
                )
                nc.gpsimd.dma_start(out=yT[:, mo, :], in_=ot[:])

            # x: fp32 [in, tok] -> bf16, pinned in SBUF (32KB/partition).
            # x chunks alternate between the two DMA paths (weights share
            # the sync path) so the streams run in parallel.
            chunk_start = [sum(CHUNKS[:i]) for i in range(NCH)]
            x_tiles = []
            load_w(0)
            w_emitted = 1
            for c in range(NCH):
                cw = CHUNKS[c]
                xt = xpool.tile([P, cw, TOK], bf16, name=f"xbf{c}")
                st = xstage.tile([P, 4, TOK], f32, name="xst")[:, :cw, :]
                eng = nc.gpsimd if c % 2 == 0 else nc.sync
                eng.dma_start(
                    out=st[:], in_=xT[:, chunk_start[c]:chunk_start[c] + cw, :])
                nc.vector.tensor_copy(out=xt[:], in_=st[:])
                x_tiles.append(xt)
                if w_emitted < SPLIT:
                    load_w(w_emitted)
                    w_emitted += 1
            for mo in range(w_emitted, SPLIT + W_PREFETCH):
                load_w(mo)

            def x_block(ko):
                for c in range(NCH):
                    if chunk_start[c] <= ko < chunk_start[c] + CHUNKS[c]:
                        return x_tiles[c][:, ko - chunk_start[c], :]
                raise AssertionError(ko)

            # Head: SPLIT open PSUM groups accumulated chunk-major, so the
            # PE consumes each x chunk the moment it lands instead of
            # stalling until the whole 8MB x transfer completes.
            ps_head = [pshead.tile([P, TOK], mybir.dt.float32, name=f"psh{m}")
                       for m in range(SPLIT)]
            for c in range(NCH):
                for mo in range(SPLIT):
                    for j in range(CHUNKS[c]):
                        ko = chunk_start[c] + j
                        nc.tensor.matmul(
                            ps_head[mo][:],
                            lhsT=wt_tiles[mo][:, ko, :],
                            rhs=x_tiles[c][:, j, :],
                            start=(ko == 0),
                            stop=(ko == KO - 1),
                        )
            for mo in range(SPLIT):
                evict(mo, ps_head[mo])
                wt_tiles[mo] = None

            # Steady state: one group per mo, K-contiguous.
            for mo in range(SPLIT, MO):
                if mo + W_PREFETCH < MO:
                    load_w(mo + W_PREFETCH)
                wt = wt_tiles[mo]
                ps = pspool.tile([P, TOK], mybir.dt.float32, name="ps")
                for ko in range(KO):
                    nc.tensor.matmul(
                        ps[:],
                        lhsT=wt[:, ko, :],
                        rhs=x_block(ko),
                        start=(ko == 0),
                        stop=(ko == KO - 1),
                    )
                wt_tiles[mo] = None
                evict(mo, ps)

    nc.compile()
    return nc


def prep_inputs(x, weight, bias):
    """Host-side shard/layout prep. Returns (in_maps, inv_scale)."""
    x = np.asarray(x, dtype=np.float32)
    weight = np.asarray(weight, dtype=np.float32)
    bias = np.asarray(bias, dtype=np.float32)

    # Quantize weights exactly as the reference does (fp32 arithmetic).
    s = np.float32(127.0) / np.max(np.abs(weight))
    wq_f = np.clip(np.trunc(weight * s), -127.0, 127.0)
    inv_scale = np.float32(1.0) / s

    # w_q^T laid out [mo, p(k), ko, q(out)] so each per-core DMA block
    # [p, ko, q] is contiguous per partition. int8 (exact): upcast on device.
    wq_i8 = wq_f.astype(np.int8)
    wq_dram = np.ascontiguousarray(
        wq_i8.reshape(MO, P, KO, P).transpose(0, 3, 2, 1)
    )

    bias_dram = np.ascontiguousarray(bias.reshape(MO, P).T)
    inv_dram = np.full((P, 1), inv_scale, dtype=np.float32)

    in_maps = []
    for c in range(N_CORES):
        x_c = x[c * TOK:(c + 1) * TOK, :]                    # [tok, in]
        xT_dram = np.ascontiguousarray(
            x_c.reshape(TOK, KO, P).transpose(2, 1, 0))      # [p, ko, tok]
        in_maps.append({
            "xT": xT_dram,
            "wq": wq_dram,
            "bias": bias_dram,
            "inv_s": inv_dram,
        })
    return in_maps


def gather_output(results):
    """results: list of per-core dicts with 'yT' [P, MO, TOK] -> y [4096, 4096]."""
    blocks = []
    for c in range(N_CORES):
        yT = results[c]["yT"]                                # [q, mo, tok]
        y_c = yT.transpose(1, 0, 2).reshape(OUT_F, TOK).T    # [tok, out]
        blocks.append(y_c)
    return np.ascontiguousarray(np.concatenate(blocks, axis=0), dtype=np.float32)


_NC_CACHE = None


def get_program():
    global _NC_CACHE
    if _NC_CACHE is None:
        _NC_CACHE = build_program(debug=False)
    return _NC_CACHE


def run(x, weight, bias, trace=False, **run_kwargs):
    from concourse.bass_utils import run_bass_kernel_spmd

    nc = get_program()
    in_maps = prep_inputs(x, weight, bias)
    res = run_bass_kernel_spmd(nc, in_maps, list(range(N_CORES)),
                               trace=trace, **run_kwargs)
    return gather_output(res.results), res


def kernel(x, weight, bias):
    y, _ = run(x, weight, bias, trace=False)
    return y
